# revision 6
# baseline (speedup 1.0000x reference)
"""Trainium2 Bass kernel for nn_Attention_86199993631321.

Reference computation (B=8, N=128, H=512):
    pair[b,i,j,:] = x[b,i,:] + x[b,j,:]
    out = pair @ W.T + b                # [B, N, N, H]

Algebra: out[b,i,j,:] = P[b,i,:] + P[b,j,:] with P = x @ W.T + 0.5*b.
Sharding: data-parallel over batch (core b handles batch b).

v2 design (vs the 79us v1 baseline, which wrote the full 16.8MB bf16
output per core and was DVE-bound in steady state):
  - The output is symmetric: only the block-lower-triangle (i_blk >=
    j_blk, 8-column blocks) is computed and written (8.7MB bf16); the
    host mirrors the strict upper block-triangle and upcasts.
  - Column blocks are packed into 128-partition PSUM tiles at 32-row
    granularity: blocks 0-3 full height; pairs (4,12),(5,13),(6,14),
    (7,15) as 96+32; pairs (8,10),(9,11) as 64+64.  10 tiles = 20
    PSUM groups of [128, 4*512] f32 instead of v1's 24.
  - Per group, the PE writes BOTH terms into PSUM: K=1 rank-1 matmuls
    broadcast P[j] rows (FD=1024, two j columns per matmul, 4 array
    quadrants concurrent), then a K=128 matmul with a host-supplied
    0/1 sigma-permutation matrix accumulates P[sigma(p)] (the i term,
    row-shifted for packed tiles).  Eviction is then a single 1x
    PSUM->SBUF bf16 copy (ACT/DVE/GPS by route).  Alternative TT
    routes skip the identity matmul and add the i term on DVE/GPS
    reading PSUM directly (in0 = sigma-shifted SBUF copy of P).
  - Output DMAs write exact triangle heights, alternating the two
    HWDGE rings (sync/scalar).
"""

import sys

if "/opt/trn_rl_repo" not in sys.path:
    sys.path.insert(0, "/opt/trn_rl_repo")

import numpy as np

B, N, H = 8, 128, 512
NCORES = 8
KC = H // 128  # contraction chunks for the P matmul
TTW = 4        # j columns per PSUM group
# packed input layout (per core, bf16): wx[h, 0:128] = x.T,
# wx[h, 128:640] = W.T, wx[0, 640:768] = 1.0 (ones row for the bias matmul)
WXW = N + H + 128
# perm input [128, 3*128] bf16: sigma matrices for ident-accumulate MMs
#   cols 0:128 identity; 128:256 sigma96 (p<96 -> p+32 else p);
#   256:384 sigma64 (p<64 -> p+64 else p)
#
# btiles: (kind, blkA, blkB); kind F: one full-height block;
# P96: A at partitions [0,96) rows 32..127, B at [96,128) rows 96..127;
# P64: A at [0,64) rows 64..127, B at [64,128) rows 64..127.
BTILES = [
    ("F", 0, None), ("F", 1, None), ("F", 2, None), ("F", 3, None),
    ("P96", 4, 12), ("P96", 5, 13), ("P96", 6, 14), ("P96", 7, 15),
    ("P64", 8, 10), ("P64", 9, 11),
]
# per-group route, 2 groups per btile, in btile order (GPSIMD has no
# PSUM port, so every route evicts via ACT or DVE):
#   IA/IV: PE sigma-ident-accumulate + {ACT, DVE} 1x copy eviction
#   TV:    DVE tensor_tensor add (in0 = sigma-P SBUF, in1 = PSUM), 1x
#   SV:    ACT 1x copy PSUM->SBUF + DVE 2x bf16 TT add
ROUTES = [
    "TV", "IA", "SV", "IA", "TV", "IV", "IA", "TV",
    "IA", "SV", "TV", "IA", "IV", "TV", "IA", "SV",
    "IA", "TV", "IA", "TV",
]
# matmul free dim is capped at 512: the f32 PSUM output of one matmul
# must stay inside a single 2KB PSUM bank (ISA check)

_BUILT = {}


def _build_nc():
    import concourse.bass as bass
    import concourse.bacc as bacc
    import concourse.tile as tile
    from concourse import mybir

    f32 = mybir.dt.float32
    bf16 = mybir.dt.bfloat16
    ADD = mybir.AluOpType.add
    COPY = mybir.ActivationFunctionType.Copy

    nc = bacc.Bacc()
    wx_ext = nc.declare_dram_parameter("wx", [H, WXW], bf16, isOutput=False)
    hb_ext = nc.declare_dram_parameter("halfb", [1, H], bf16, isOutput=False)
    perm_ext = nc.declare_dram_parameter("perm", [128, 3 * 128], bf16, isOutput=False)
    out_ext = nc.declare_dram_parameter("out", [N, N, H], bf16, isOutput=True)

    group_idx = [0]
    dma_idx = [0]

    with tile.TileContext(nc) as tc:
        with (
            tc.tile_pool(name="const", bufs=1) as const,
            tc.tile_pool(name="stage", bufs=6) as stage,
            tc.tile_pool(name="outp", bufs=6) as outp,
            tc.tile_pool(name="psum", bufs=2, space="PSUM") as psum,
        ):
            # ---- load packed inputs ----
            wx_sb = const.tile([128, KC, WXW], bf16)  # [h_local, (kc, m)]
            wx_v = wx_ext.rearrange("(c p) m -> p c m", p=128)
            for c in range(KC):
                eng = nc.sync if c % 2 == 0 else nc.scalar
                eng.dma_start(out=wx_sb[:, c, :], in_=wx_v[:, c, :])
            perm_sb = const.tile([128, 3 * 128], bf16)
            nc.sync.dma_start(out=perm_sb, in_=perm_ext[:, :])
            ones_sb = const.tile([128, 128], bf16)
            nc.vector.memset(ones_sb, 1.0)
            hb_sb = const.tile([1, H], bf16)
            nc.gpsimd.dma_start(out=hb_sb, in_=hb_ext[:, :])

            # ---- P = x @ W.T + 0.5*b -> PSUM [128(i), 512(o)] ----
            ps_proj = psum.tile([128, TTW * H], f32, tag="ps")
            for c in range(KC):
                nc.tensor.matmul(
                    ps_proj[:, 0:H],
                    wx_sb[:, c, 0:N],
                    wx_sb[:, c, N : N + H],
                    start=(c == 0),
                    stop=False,
                )
            nc.tensor.matmul(
                ps_proj[:, 0:H],
                wx_sb[0:1, 0, N + H : N + H + 128],
                hb_sb,
                start=False,
                stop=True,
            )

            # ---- P replicated 4x along free dim (bf16) ----
            P_rep = const.tile([128, TTW, H], bf16)
            nc.scalar.activation(P_rep[:, 0, :], ps_proj[:, 0:H], COPY)
            nc.vector.tensor_copy(P_rep[:, 1, :], P_rep[:, 0, :])
            nc.vector.tensor_copy(P_rep[:, 2:4, :], P_rep[:, 0:2, :])

            # sigma-shifted replicated copies for TT routes on packed tiles
            P_g96 = const.tile([128, TTW, H], bf16)
            P_g64 = const.tile([128, TTW, H], bf16)
            nc.scalar.dma_start(out=P_g96[0:96, :, :], in_=P_rep[32:128, :, :])
            nc.scalar.dma_start(out=P_g96[96:128, :, :], in_=P_rep[96:128, :, :])
            nc.scalar.dma_start(out=P_g64[0:64, :, :], in_=P_rep[64:128, :, :])
            nc.scalar.dma_start(out=P_g64[64:128, :, :], in_=P_rep[64:128, :, :])
            SIG = {"F": P_rep, "P96": P_g96, "P64": P_g64}
            POFF = {"F": 0, "P96": 128, "P64": 256}

            def stage_chunk(j0):
                # quadrant q (partition 32q) holds rows j0+2q, j0+2q+1
                chunk = stage.tile([128, 2 * H], bf16, name=f"ch_{j0}", tag="chunk")
                nc.gpsimd.dma_start(
                    out=chunk[0:128:32, :],
                    in_=P_rep[j0 : j0 + 8, 0, :],
                )
                return chunk

            def next_route():
                r = ROUTES[group_idx[0] % len(ROUTES)]
                group_idx[0] += 1
                return r

            def out_dma(dst, src):
                eng = nc.sync if dma_idx[0] % 2 == 0 else nc.scalar
                dma_idx[0] += 1
                eng.dma_start(out=dst, in_=src)

            def do_btile(kind, kA, kB):
                # segs: (chunk, part_lo, part_hi, row_shift)
                if kind == "F":
                    segs = [(stage_chunk(8 * kA), 0, 128, 0)]
                elif kind == "P96":
                    segs = [
                        (stage_chunk(8 * kA), 0, 96, 32),
                        (stage_chunk(8 * kB), 96, 128, 0),
                    ]
                else:
                    segs = [
                        (stage_chunk(8 * kA), 0, 64, 64),
                        (stage_chunk(8 * kB), 64, 128, 0),
                    ]
                sig = SIG[kind]
                poff = POFF[kind]
                out_tile = outp.tile([128, 8, H], bf16, name="ot")
                for g in range(2):
                    route = next_route()
                    ident = route[0] == "I"
                    ps = psum.tile([128, TTW * H], f32, tag="ps", name=f"ps{g}")
                    for up in range(2):
                        q = 2 * g + up
                        for s in range(2):
                            u = 2 * up + s
                            for chunk, plo, phi, _sh in segs:
                                nc.tensor.matmul(
                                    ps[plo:phi, u * H : (u + 1) * H],
                                    ones_sb[32 * q : 32 * q + 1, 0 : phi - plo],
                                    chunk[32 * q : 32 * q + 1, s * H : (s + 1) * H],
                                    start=True,
                                    stop=not ident,
                                    tile_position=(32 * q, plo),
                                )
                    if ident:
                        for u in range(TTW):
                            nc.tensor.matmul(
                                ps[:, u * H : (u + 1) * H],
                                perm_sb[:, poff : poff + 128],
                                P_rep[:, u, :],
                                start=False,
                                stop=True,
                                tile_position=(0, 0),
                            )
                    ps_v = ps.rearrange("p (u h) -> p u h", u=TTW)
                    out_sl = out_tile[:, g * TTW : (g + 1) * TTW, :]
                    if route == "IA":
                        nc.scalar.activation(out_sl, ps_v, COPY)
                    elif route == "IV":
                        nc.vector.tensor_copy(out_sl, ps_v)
                    elif route == "TV":
                        nc.vector.tensor_tensor(
                            out=out_sl, in0=sig[:, :, :], in1=ps_v, op=ADD
                        )
                    else:  # SV
                        bc = stage.tile([128, TTW, H], bf16, name="bc", tag="bc")
                        nc.scalar.activation(bc, ps_v, COPY)
                        nc.vector.tensor_tensor(
                            out=out_sl, in0=sig[:, :, :], in1=bc, op=ADD
                        )
                # exact-height writes, one per seg
                for _chunk, plo, phi, sh in segs:
                    blk = kA if plo == 0 else kB
                    plo_eff = max(plo, 8 * blk - sh)
                    r0 = plo_eff + sh
                    out_dma(
                        out_ext[r0:128, 8 * blk : 8 * blk + 8, :],
                        out_tile[plo_eff:phi, :, :],
                    )

            for kind, kA, kB in BTILES:
                do_btile(kind, kA, kB)
    nc.compile()
    return nc


def _get_nc():
    if "nc" not in _BUILT:
        _BUILT["nc"] = _build_nc()
    return _BUILT["nc"]


def _make_perm():
    perm = np.zeros((128, 3 * 128), dtype=np.float32)
    p = np.arange(128)
    perm[p, p] = 1.0
    s96 = np.where(p < 96, p + 32, p)
    perm[s96, 128 + p] = 1.0
    s64 = np.where(p < 64, p + 64, p)
    perm[s64, 256 + p] = 1.0
    return perm


def _make_in_maps(local_feats, W, b):
    import ml_dtypes

    bf = ml_dtypes.bfloat16
    local_feats = np.asarray(local_feats, dtype=np.float32)
    W = np.asarray(W, dtype=np.float32)
    b = np.asarray(b, dtype=np.float32)
    hb = np.ascontiguousarray((0.5 * b).reshape(1, H)).astype(bf)
    perm = _make_perm().astype(bf)
    base = np.zeros((H, WXW), dtype=np.float32)
    base[:, N : N + H] = W.T
    base[0, N + H :] = 1.0
    in_maps = []
    for c in range(NCORES):
        wx = base.copy()
        wx[:, :N] = local_feats[c].T
        in_maps.append({"wx": wx.astype(bf), "halfb": hb, "perm": perm})
    return in_maps


def _collect(res):
    iu, ju = np.triu_indices(16, 1)
    full = np.empty((NCORES, N, N, H), dtype=np.float32)
    for c in range(NCORES):
        a = np.asarray(res.results[c]["out"]).astype(np.float32)
        v = a.reshape(16, 8, 16, 8, H)
        v[iu, :, ju] = v[ju, :, iu].swapaxes(1, 2)
        full[c] = a
    return full


def kernel(local_feats, W, b):
    from concourse.bass_utils import run_bass_kernel_spmd

    nc = _get_nc()
    in_maps = _make_in_maps(local_feats, W, b)
    res = run_bass_kernel_spmd(nc, in_maps, core_ids=list(range(NCORES)))
    return _collect(res)


def run_profiled(local_feats, W, b, **trace_kwargs):
    """Like kernel() but with neuron-profile tracing; returns (out, results)."""
    from concourse.bass_utils import run_bass_kernel_spmd

    nc = _get_nc()
    in_maps = _make_in_maps(local_feats, W, b)
    res = run_bass_kernel_spmd(
        nc, in_maps, core_ids=list(range(NCORES)), trace=True, **trace_kwargs
    )
    return _collect(res), res


# revision 8
# speedup vs baseline: 1.0809x; 1.0809x over previous
"""Trainium2 Bass kernel for nn_Attention_86199993631321.

Reference computation (B=8, N=128, H=512):
    pair[b,i,j,:] = x[b,i,:] + x[b,j,:]
    out = pair @ W.T + b                # [B, N, N, H]

Algebra: out[b,i,j,:] = P[b,i,:] + P[b,j,:] with P = x @ W.T + 0.5*b.
Sharding: data-parallel over batch (core b handles batch b).

v2 design (vs the 79us v1 baseline, which wrote the full 16.8MB bf16
output per core and was DVE-bound in steady state):
  - The output is symmetric: only the block-lower-triangle (i_blk >=
    j_blk, 8-column blocks) is computed and written (8.7MB bf16); the
    host mirrors the strict upper block-triangle and upcasts.
  - Column blocks are packed into 128-partition PSUM tiles at 32-row
    granularity: blocks 0-3 full height; pairs (4,12),(5,13),(6,14),
    (7,15) as 96+32; pairs (8,10),(9,11) as 64+64.  10 tiles = 20
    PSUM groups of [128, 4*512] f32 instead of v1's 24.
  - Per group, the PE writes BOTH terms into PSUM: K=1 rank-1 matmuls
    broadcast P[j] rows (FD=1024, two j columns per matmul, 4 array
    quadrants concurrent), then a K=128 matmul with a host-supplied
    0/1 sigma-permutation matrix accumulates P[sigma(p)] (the i term,
    row-shifted for packed tiles).  Eviction is then a single 1x
    PSUM->SBUF bf16 copy (ACT/DVE/GPS by route).  Alternative TT
    routes skip the identity matmul and add the i term on DVE/GPS
    reading PSUM directly (in0 = sigma-shifted SBUF copy of P).
  - Output DMAs write exact triangle heights, alternating the two
    HWDGE rings (sync/scalar).
"""

import sys

if "/opt/trn_rl_repo" not in sys.path:
    sys.path.insert(0, "/opt/trn_rl_repo")

import numpy as np

B, N, H = 8, 128, 512
NCORES = 8
KC = H // 128  # contraction chunks for the P matmul
TTW = 4        # j columns per PSUM group
# packed input layout (per core, bf16): wx[h, 0:128] = x.T,
# wx[h, 128:640] = W.T, wx[0, 640:768] = 1.0 (ones row for the bias matmul)
WXW = N + H + 128
# perm input [128, 3*128] bf16: sigma matrices for ident-accumulate MMs
#   cols 0:128 identity; 128:256 sigma96 (p<96 -> p+32 else p);
#   256:384 sigma64 (p<64 -> p+64 else p)
#
# btiles: (kind, blkA, blkB); kind F: one full-height block;
# P96: A at partitions [0,96) rows 32..127, B at [96,128) rows 96..127;
# P64: A at [0,64) rows 64..127, B at [64,128) rows 64..127.
BTILES = [
    ("F", 0, None), ("F", 1, None), ("F", 2, None), ("F", 3, None),
    ("P96", 4, 12), ("P96", 5, 13), ("P96", 6, 14), ("P96", 7, 15),
    ("P64", 8, 10), ("P64", 9, 11),
]
# per-group route, 2 groups per btile, in btile order (GPSIMD has no
# PSUM port, so every route evicts via ACT or DVE; the PE runs at a
# fixed 1.2 GHz here, so full-array ident matmuls are expensive and
# used sparingly):
#   IA/IV: PE sigma-ident-accumulate + {ACT, DVE} 1x copy eviction
#   TV:    DVE tensor_tensor add (in0 = sigma-P SBUF, in1 = PSUM), 1x
#   SV:    ACT 1x copy PSUM->SBUF + DVE 2x bf16 TT add
#   SG:    ACT 1x copy PSUM->SBUF + GPSIMD TT add
ROUTES = [
    "TV", "SG", "SV", "TV", "SG", "IA", "TV", "SV",
    "SG", "TV", "IA", "SV", "TV", "SG", "TV", "IA",
    "SG", "TV", "SV", "TV",
]
# matmul free dim is capped at 512: the f32 PSUM output of one matmul
# must stay inside a single 2KB PSUM bank (ISA check)

_BUILT = {}


def _build_nc():
    import concourse.bass as bass
    import concourse.bacc as bacc
    import concourse.tile as tile
    from concourse import mybir

    f32 = mybir.dt.float32
    bf16 = mybir.dt.bfloat16
    ADD = mybir.AluOpType.add
    COPY = mybir.ActivationFunctionType.Copy

    nc = bacc.Bacc()
    wx_ext = nc.declare_dram_parameter("wx", [H, WXW], bf16, isOutput=False)
    hb_ext = nc.declare_dram_parameter("halfb", [1, H], bf16, isOutput=False)
    perm_ext = nc.declare_dram_parameter("perm", [128, 3 * 128], bf16, isOutput=False)
    out_ext = nc.declare_dram_parameter("out", [N, N, H], bf16, isOutput=True)

    group_idx = [0]
    dma_idx = [0]

    with tile.TileContext(nc) as tc:
        with (
            tc.tile_pool(name="const", bufs=1) as const,
            tc.tile_pool(name="stage", bufs=6) as stage,
            tc.tile_pool(name="outp", bufs=6) as outp,
            tc.tile_pool(name="psum", bufs=2, space="PSUM") as psum,
        ):
            # ---- load packed inputs ----
            wx_sb = const.tile([128, KC, WXW], bf16)  # [h_local, (kc, m)]
            wx_v = wx_ext.rearrange("(c p) m -> p c m", p=128)
            for c in range(KC):
                eng = nc.sync if c % 2 == 0 else nc.scalar
                eng.dma_start(out=wx_sb[:, c, :], in_=wx_v[:, c, :])
            perm_sb = const.tile([128, 3 * 128], bf16)
            nc.sync.dma_start(out=perm_sb, in_=perm_ext[:, :])
            ones_sb = const.tile([128, 128], bf16)
            nc.vector.memset(ones_sb, 1.0)
            hb_sb = const.tile([1, H], bf16)
            nc.gpsimd.dma_start(out=hb_sb, in_=hb_ext[:, :])

            # ---- P = x @ W.T + 0.5*b -> PSUM [128(i), 512(o)] ----
            ps_proj = psum.tile([128, TTW * H], f32, tag="ps")
            for c in range(KC):
                nc.tensor.matmul(
                    ps_proj[:, 0:H],
                    wx_sb[:, c, 0:N],
                    wx_sb[:, c, N : N + H],
                    start=(c == 0),
                    stop=False,
                )
            nc.tensor.matmul(
                ps_proj[:, 0:H],
                wx_sb[0:1, 0, N + H : N + H + 128],
                hb_sb,
                start=False,
                stop=True,
            )

            # ---- P replicated 4x along free dim (bf16) ----
            P_rep = const.tile([128, TTW, H], bf16)
            nc.scalar.activation(P_rep[:, 0, :], ps_proj[:, 0:H], COPY)
            nc.vector.tensor_copy(P_rep[:, 1, :], P_rep[:, 0, :])
            nc.vector.tensor_copy(P_rep[:, 2:4, :], P_rep[:, 0:2, :])

            # sigma-shifted replicated copies for TT routes on packed tiles
            P_g96 = const.tile([128, TTW, H], bf16)
            P_g64 = const.tile([128, TTW, H], bf16)
            nc.scalar.dma_start(out=P_g96[0:96, :, :], in_=P_rep[32:128, :, :])
            nc.scalar.dma_start(out=P_g96[96:128, :, :], in_=P_rep[96:128, :, :])
            nc.scalar.dma_start(out=P_g64[0:64, :, :], in_=P_rep[64:128, :, :])
            nc.scalar.dma_start(out=P_g64[64:128, :, :], in_=P_rep[64:128, :, :])
            SIG = {"F": P_rep, "P96": P_g96, "P64": P_g64}
            POFF = {"F": 0, "P96": 128, "P64": 256}

            def stage_chunk(j0):
                # quadrant q (partition 32q) holds rows j0+2q, j0+2q+1
                chunk = stage.tile([128, 2 * H], bf16, name=f"ch_{j0}", tag="chunk")
                nc.gpsimd.dma_start(
                    out=chunk[0:128:32, :],
                    in_=P_rep[j0 : j0 + 8, 0, :],
                )
                return chunk

            def next_route():
                r = ROUTES[group_idx[0] % len(ROUTES)]
                group_idx[0] += 1
                return r

            def out_dma(dst, src):
                eng = nc.sync if dma_idx[0] % 2 == 0 else nc.scalar
                dma_idx[0] += 1
                eng.dma_start(out=dst, in_=src)

            def do_btile(kind, kA, kB):
                # segs: (chunk, part_lo, part_hi, row_shift)
                if kind == "F":
                    segs = [(stage_chunk(8 * kA), 0, 128, 0)]
                elif kind == "P96":
                    segs = [
                        (stage_chunk(8 * kA), 0, 96, 32),
                        (stage_chunk(8 * kB), 96, 128, 0),
                    ]
                else:
                    segs = [
                        (stage_chunk(8 * kA), 0, 64, 64),
                        (stage_chunk(8 * kB), 64, 128, 0),
                    ]
                sig = SIG[kind]
                poff = POFF[kind]
                out_tile = outp.tile([128, 8, H], bf16, name="ot")
                for g in range(2):
                    route = next_route()
                    ident = route[0] == "I"
                    ps = psum.tile([128, TTW * H], f32, tag="ps", name=f"ps{g}")
                    for up in range(2):
                        q = 2 * g + up
                        for s in range(2):
                            u = 2 * up + s
                            for chunk, plo, phi, _sh in segs:
                                nc.tensor.matmul(
                                    ps[plo:phi, u * H : (u + 1) * H],
                                    ones_sb[32 * q : 32 * q + 1, 0 : phi - plo],
                                    chunk[32 * q : 32 * q + 1, s * H : (s + 1) * H],
                                    start=True,
                                    stop=not ident,
                                    tile_position=(32 * q, plo),
                                )
                    if ident:
                        for u in range(TTW):
                            nc.tensor.matmul(
                                ps[:, u * H : (u + 1) * H],
                                perm_sb[:, poff : poff + 128],
                                P_rep[:, u, :],
                                start=False,
                                stop=True,
                                tile_position=(0, 0),
                            )
                    ps_v = ps.rearrange("p (u h) -> p u h", u=TTW)
                    out_sl = out_tile[:, g * TTW : (g + 1) * TTW, :]
                    if route == "IA":
                        nc.scalar.activation(out_sl, ps_v, COPY)
                    elif route == "IV":
                        nc.vector.tensor_copy(out_sl, ps_v)
                    elif route == "TV":
                        nc.vector.tensor_tensor(
                            out=out_sl, in0=sig[:, :, :], in1=ps_v, op=ADD
                        )
                    else:  # SV / SG
                        bc = stage.tile([128, TTW, H], bf16, name="bc", tag="bc")
                        nc.scalar.activation(bc, ps_v, COPY)
                        eng = nc.vector if route == "SV" else nc.gpsimd
                        eng.tensor_tensor(
                            out=out_sl, in0=sig[:, :, :], in1=bc, op=ADD
                        )
                # exact-height writes, one per seg
                for _chunk, plo, phi, sh in segs:
                    blk = kA if plo == 0 else kB
                    plo_eff = max(plo, 8 * blk - sh)
                    r0 = plo_eff + sh
                    out_dma(
                        out_ext[r0:128, 8 * blk : 8 * blk + 8, :],
                        out_tile[plo_eff:phi, :, :],
                    )

            for kind, kA, kB in BTILES:
                do_btile(kind, kA, kB)
    nc.compile()
    return nc


def _get_nc():
    if "nc" not in _BUILT:
        _BUILT["nc"] = _build_nc()
    return _BUILT["nc"]


def _make_perm():
    perm = np.zeros((128, 3 * 128), dtype=np.float32)
    p = np.arange(128)
    perm[p, p] = 1.0
    s96 = np.where(p < 96, p + 32, p)
    perm[s96, 128 + p] = 1.0
    s64 = np.where(p < 64, p + 64, p)
    perm[s64, 256 + p] = 1.0
    return perm


def _make_in_maps(local_feats, W, b):
    import ml_dtypes

    bf = ml_dtypes.bfloat16
    local_feats = np.asarray(local_feats, dtype=np.float32)
    W = np.asarray(W, dtype=np.float32)
    b = np.asarray(b, dtype=np.float32)
    hb = np.ascontiguousarray((0.5 * b).reshape(1, H)).astype(bf)
    perm = _make_perm().astype(bf)
    base = np.zeros((H, WXW), dtype=np.float32)
    base[:, N : N + H] = W.T
    base[0, N + H :] = 1.0
    in_maps = []
    for c in range(NCORES):
        wx = base.copy()
        wx[:, :N] = local_feats[c].T
        in_maps.append({"wx": wx.astype(bf), "halfb": hb, "perm": perm})
    return in_maps


def _collect(res):
    iu, ju = np.triu_indices(16, 1)
    full = np.empty((NCORES, N, N, H), dtype=np.float32)
    for c in range(NCORES):
        a = np.asarray(res.results[c]["out"]).astype(np.float32)
        v = a.reshape(16, 8, 16, 8, H)
        v[iu, :, ju] = v[ju, :, iu].swapaxes(1, 2)
        full[c] = a
    return full


def kernel(local_feats, W, b):
    from concourse.bass_utils import run_bass_kernel_spmd

    nc = _get_nc()
    in_maps = _make_in_maps(local_feats, W, b)
    res = run_bass_kernel_spmd(nc, in_maps, core_ids=list(range(NCORES)))
    return _collect(res)


def run_profiled(local_feats, W, b, **trace_kwargs):
    """Like kernel() but with neuron-profile tracing; returns (out, results)."""
    from concourse.bass_utils import run_bass_kernel_spmd

    nc = _get_nc()
    in_maps = _make_in_maps(local_feats, W, b)
    res = run_bass_kernel_spmd(
        nc, in_maps, core_ids=list(range(NCORES)), trace=True, **trace_kwargs
    )
    return _collect(res), res
